# revision 7
# baseline (speedup 1.0000x reference)
"""Distributed GRACE-style contrastive loss on 8 Trainium2 NeuronCores.

Math (reference):
    h = elu(z @ W1 + b1) @ W2 + b2           for z1, z2    -> h1, h2
    hn = h / max(||h||_row, eps)
    With H = [h1n; h2n] (2N x D) and G = H H^T (symmetric, 2N x 2N):
      denom1_i = sum_j exp(2 G[i, j])     - e^2   (row i < N)
      denom2_i = sum_j exp(2 G[N+i, j])   - e^2
      loss = mean_i [ 0.5 (log denom1_i + log denom2_i) - 2 (h1n_i . h2n_i) ]

Strategy: G is symmetric, so only the upper triangle of its 16x16 grid of
1024x1024 blocks (136 blocks) is computed; each off-diagonal block yields
both a row-sum (DVE reduce of exp) and a column-sum (Pool-engine
accumulate + ones-matmul partition reduce) so every exp element serves
two output rows.  A circulant cover assigns exactly 17 blocks per core --
core k computes (k, k+d) and (k+8, (k+8+d) mod 16) for d = 1..7 plus the
three blocks touching only local data -- so the stationary matmul operand
is always the core's own fp8 tile and only the moving-tile DMA source
offsets depend on the rank (computed at runtime from partition_id).
Projection runs transposed (features on partitions) with fp8 DoubleRow
matmuls, biases folded in via rank-1 bias matmuls so the elu/normalize
chain reads PSUM directly.  Both normalized tensors AllGather into one
16-slot shared buffer (slot = global row block).  Final log/mean combine
runs on the host from small per-core outputs.
"""

import sys

sys.path.insert(0, "/opt/trn_rl_repo")

import numpy as np
from concourse import bacc, mybir, tile
from concourse.bass_utils import run_bass_kernel_spmd

F32 = mybir.dt.float32
BF16 = mybir.dt.bfloat16
FP8 = mybir.dt.float8e4
AF = mybir.ActivationFunctionType
ALU = mybir.AluOpType
DR = mybir.MatmulPerfMode.DoubleRow

N = 8192          # total rows per tensor
D = 512           # hidden dim (= proj dim)
NCORES = 8
NL = N // NCORES  # 1024 local rows per core per tensor
TAU = 0.5
SIGMA = 16.0      # fp8 pre-scale; S accumulates SIGMA^2 * S_true
SCALE_DEV = (1.0 / TAU) / (SIGMA * SIGMA)  # exp scale on device
NQ = 2            # two K=256 DoubleRow groups
NDC = D // 128    # 4 feature chunks of 128 partitions
NIT = NL // 128   # 8 row tiles of 128 per block
NSLOT = 16        # gathered row blocks (2 tensors x 8 cores)
EPS = 1e-12

# Per-core block schedule; identical structure on every core, the moving
# slot for dyn entries is rank-relative: dynA -> slot rank+d, dynB ->
# slot (rank+d+8) mod 16.  (stat_t, moving, has_colsum)
SCHED = [
    (0, ("local", 0), False),   # (k, k)       diagonal
    (1, ("local", 1), False),   # (k+8, k+8)   diagonal
    (0, ("local", 1), True),    # (k, k+8)     S12 diagonal block
]
for _d in range(1, 8):
    SCHED.append((0, ("dynA", _d), True))   # (k, k+d)
    SCHED.append((1, ("dynB", _d), True))   # (k+8, (k+8+d) mod 16)
NBLK = len(SCHED)                       # 17
NCS = sum(1 for s in SCHED if s[2])     # 15

_CACHE = {}


def _build():
    nc = bacc.Bacc("TRN2", target_bir_lowering=False, debug=False,
                   num_devices=NCORES)

    # ---- I/O ----------------------------------------------------------
    # z comes pre-packed in fp8 DoubleRow pair layout [q, p, pair, cols]
    # with contraction index d = q*256 + pair*128 + p
    z_d = [nc.declare_dram_parameter(f"z{t}f8", [NQ, 128, NQ, NL], FP8,
                                     isOutput=False) for t in range(2)]
    w1_d = nc.declare_dram_parameter("w1f8", [NQ, 128, NQ, D], FP8,
                                     isOutput=False)
    w2_d = nc.declare_dram_parameter("w2f8", [NQ, 128, NQ, D], FP8,
                                     isOutput=False)
    # bias rows (x16, bf16) for the rank-1 bias matmuls; b2 adjusted on
    # host: b2 - colsum(W2) (folds the elu()+1 shift back out)
    b1_d = nc.declare_dram_parameter("b1r", [1, D], BF16, isOutput=False)
    b2_d = nc.declare_dram_parameter("b2r", [1, D], BF16, isOutput=False)

    out_rs = nc.declare_dram_parameter("out_rs", [128, NBLK * NIT], F32,
                                       isOutput=True)
    out_cs = nc.declare_dram_parameter("out_cs", [1, NCS * NL], F32,
                                       isOutput=True)
    out_diag = nc.declare_dram_parameter("out_diag", [1, NL], F32,
                                         isOutput=True)

    with tile.TileContext(nc) as tc:
        with (
            tc.tile_pool(name="const", bufs=1) as constp,
            tc.tile_pool(name="locals", bufs=1) as localp,
            tc.tile_pool(name="accs", bufs=1) as accp,
            tc.tile_pool(name="dram", bufs=1, space="DRAM") as dramp,
        ):
            ones_col_bf = constp.tile([128, 1], BF16)
            nc.vector.memset(ones_col_bf[:], 1.0)
            ones_row_b = constp.tile([1, 512], BF16)
            nc.vector.memset(ones_row_b[:], 1.0)

            w1_sb = []
            w2_sb = []
            for q in range(NQ):
                w1t = constp.tile([128, NQ, D], FP8, name=f"w1_{q}")
                nc.sync.dma_start(w1t[:], w1_d[q])
                w1_sb.append(w1t)
                w2t = constp.tile([128, NQ, D], FP8, name=f"w2_{q}")
                nc.sync.dma_start(w2t[:], w2_d[q])
                w2_sb.append(w2t)
            b1_sb = constp.tile([1, D], BF16)
            nc.gpsimd.dma_start(b1_sb[:], b1_d[:])
            b2_sb = constp.tile([1, D], BF16)
            nc.gpsimd.dma_start(b2_sb[:], b2_d[:])

            # normalized local embeddings: bf16 [d, i] for the exact S12
            # diagonal, and x16-scaled fp8 in DoubleRow pair layout
            lns = [[localp.tile([128, NL], BF16, name=f"ln{t}_{dc}")
                    for dc in range(NDC)] for t in range(2)]
            lf8 = [[localp.tile([128, NQ, NL], FP8, name=f"lf8_{t}_{q}")
                    for q in range(NQ)] for t in range(2)]

            ccin = [dramp.tile([NQ, 128, NQ, NL], FP8, name=f"ccin{t}")
                    for t in range(2)]
            ccg = [dramp.tile([NCORES, NQ, 128, NQ, NL], FP8,
                              addr_space="Shared", name=f"ccg{t}")
                   for t in range(2)]

            rs_sb = accp.tile([128, NBLK * NIT], F32)
            diag_sb = accp.tile([1, NL], F32)

            # ---- Phase A: projection + normalize ----------------------
            with (
                tc.tile_pool(name="zpool", bufs=2) as zp,
                tc.tile_pool(name="elupool", bufs=2) as elup,
                tc.tile_pool(name="hpool", bufs=2) as hp,
                tc.tile_pool(name="epool", bufs=6) as ep2,
                tc.tile_pool(name="rnpool", bufs=2) as rnp,
                tc.tile_pool(name="pa_ps", bufs=3, space="PSUM") as pp2,
                tc.tile_pool(name="pa_psn", bufs=1, space="PSUM") as psn,
            ):
                for t in range(2):
                    zq = []
                    for q in range(NQ):
                        z = zp.tile([128, NQ, NL], FP8, tag=f"z{q}",
                                    name=f"z{t}{q}")
                        nc.sync.dma_start(z[:], z_d[t][q])
                        zq.append(z)
                    elus = [elup.tile([128, NQ, NL], FP8, tag=f"el{q}",
                                      name=f"el{t}{q}") for q in range(NQ)]
                    ps_n = psn.tile([1, 2, 512], F32, tag="psn",
                                    name=f"psn{t}")
                    for ih in range(2):
                        isl = slice(ih * 512, ih * 512 + 512)
                        # layer 1 + exact elu+1 from PSUM:
                        # elu(x)+1 = relu(x) + min(exp(x), 1)
                        for q in range(NQ):
                            ps_a = pp2.tile([128, 2, 512], F32, tag="ps",
                                            name=f"psa{t}{ih}{q}")
                            for pair in range(2):
                                pc = q * 2 + pair
                                csl = slice(pc * 128, pc * 128 + 128)
                                nc.tensor.matmul(ps_a[:, pair, :],
                                                 b1_sb[:, csl],
                                                 ones_row_b[:],
                                                 start=True, stop=False)
                                for qq in range(NQ):
                                    nc.tensor.matmul(
                                        ps_a[:, pair, :],
                                        w1_sb[qq][:, :, csl],
                                        zq[qq][:, :, isl],
                                        start=False, stop=qq == NQ - 1,
                                        perf_mode=DR)
                            e_sb = ep2.tile([128, 2, 512], BF16, tag="pa",
                                            name=f"e{t}{ih}{q}")
                            nc.scalar.activation(e_sb[:], ps_a[:], AF.Exp,
                                                 scale=1.0 / SIGMA)
                            r_sb = ep2.tile([128, 2, 512], BF16, tag="pa",
                                            name=f"r{t}{ih}{q}")
                            nc.vector.tensor_scalar(
                                r_sb[:], ps_a[:], 0.0, 1.0 / SIGMA,
                                op0=ALU.max, op1=ALU.mult)
                            m1 = ep2.tile([128, 2, 512], BF16, tag="pa",
                                          name=f"m{t}{ih}{q}")
                            nc.vector.tensor_scalar(m1[:], e_sb[:], 1.0,
                                                    None, op0=ALU.min)
                            nc.vector.tensor_tensor(elus[q][:, :, isl],
                                                    r_sb[:], m1[:],
                                                    op=ALU.add)
                        # layer 2 + row norms
                        hts = []
                        for ocp in range(2):
                            ps_h = pp2.tile([128, 2, 512], F32, tag="ps",
                                            name=f"psh{t}{ih}{ocp}")
                            for pair in range(2):
                                oc = ocp * 2 + pair
                                csl = slice(oc * 128, oc * 128 + 128)
                                nc.tensor.matmul(ps_h[:, pair, :],
                                                 b2_sb[:, csl],
                                                 ones_row_b[:],
                                                 start=True, stop=False)
                                for qq in range(NQ):
                                    nc.tensor.matmul(
                                        ps_h[:, pair, :],
                                        w2_sb[qq][:, :, csl],
                                        elus[qq][:, :, isl],
                                        start=False, stop=qq == NQ - 1,
                                        perf_mode=DR)
                            ht = hp.tile([128, 2, 512], BF16,
                                         tag=f"h{ocp}", name=f"h{t}{ih}{ocp}")
                            nc.scalar.activation(ht[:], ps_h[:],
                                                 AF.Identity,
                                                 scale=1.0 / SIGMA)
                            sq = ep2.tile([128, 2, 512], BF16, tag="pa",
                                          name=f"sq{t}{ih}{ocp}")
                            nc.vector.tensor_tensor(sq[:], ht[:], ht[:],
                                                    op=ALU.mult)
                            for pair in range(2):
                                nc.tensor.matmul(
                                    ps_n[0:1, ih, :], ones_col_bf[:],
                                    sq[:, pair, :],
                                    start=ocp == 0 and pair == 0,
                                    stop=ocp == 1 and pair == 1)
                            hts.append(ht)
                        # 1/max(||h||, eps) per column, broadcast via
                        # rank-1 matmul
                        nm = rnp.tile([1, 512], F32, tag="nm",
                                      name=f"nm{t}{ih}")
                        nc.scalar.activation(nm[:], ps_n[0:1, ih, :],
                                             AF.Sqrt)
                        nm2 = rnp.tile([1, 512], F32, tag="nm2",
                                       name=f"nm2{t}{ih}")
                        nc.vector.tensor_scalar(nm2[:], nm[:], EPS, None,
                                                op0=ALU.max)
                        rn32 = rnp.tile([1, 512], F32, tag="rn32",
                                        name=f"rn32{t}{ih}")
                        nc.vector.reciprocal(rn32[:], nm2[:])
                        rn = rnp.tile([1, 512], BF16, tag="rn",
                                      name=f"rn{t}{ih}")
                        nc.vector.tensor_copy(rn[:], rn32[:])
                        ps_rb = pp2.tile([128, 2, 512], F32, tag="ps",
                                         name=f"psrb{t}{ih}")
                        nc.tensor.matmul(ps_rb[:, 0, :],
                                         ones_row_b[:, 0:128], rn[:],
                                         start=True, stop=True)
                        for oc in range(NDC):
                            ocp, pair = divmod(oc, 2)
                            nc.vector.tensor_tensor(
                                lns[t][oc][:, isl],
                                hts[ocp][:, pair, :], ps_rb[:, 0, :],
                                op=ALU.mult)
                            nc.vector.tensor_scalar(
                                lf8[t][oc // 2][:, oc % 2, isl],
                                lns[t][oc][:, isl], SIGMA, None,
                                op0=ALU.mult)
                    for q in range(NQ):
                        nc.sync.dma_start(ccin[t][q], lf8[t][q][:])
                    nc.gpsimd.collective_compute(
                        "AllGather", ALU.bypass,
                        replica_groups=[list(range(NCORES))],
                        ins=[ccin[t].opt()],
                        outs=[ccg[t].opt()],
                    )

                # ---- diag12[i] = h1n_i . h2n_i (local, bf16 exact) ----
                ps_d = psn.tile([1, 2, 512], F32, tag="psn", name="psd")
                for ihx in range(2):
                    isl = slice(ihx * 512, ihx * 512 + 512)
                    for dc in range(NDC):
                        pr = ep2.tile([128, 512], BF16, tag="pr",
                                      name=f"pr{ihx}{dc}")
                        nc.vector.tensor_tensor(pr[:], lns[0][dc][:, isl],
                                                lns[1][dc][:, isl],
                                                op=ALU.mult)
                        nc.tensor.matmul(ps_d[0:1, ihx, :], ones_col_bf[:],
                                         pr[:], start=dc == 0,
                                         stop=dc == NDC - 1)
                    nc.vector.tensor_copy(diag_sb[:, isl],
                                          ps_d[0:1, ihx, :])
                nc.sync.dma_start(out_diag[:, :], diag_sb[:])

            # ---- Phase C: 17 upper-triangle blocks of G ---------------
            with (
                tc.tile_pool(name="gpool", bufs=2) as gp,
                tc.tile_pool(name="ce", bufs=4) as ep,
                tc.tile_pool(name="pc_ps", bufs=2, space="PSUM") as pp,
            ):
                rank = nc.sync.partition_id()
                cs_slot = 0
                for j, (st, mv, has_cs) in enumerate(SCHED):
                    if mv[0] == "local":
                        g = lf8[mv[1]]
                    else:
                        d = mv[1]
                        if mv[0] == "dynA":
                            slotv = rank + d
                        else:
                            slotv = (rank + d + 8) % 16
                        # slot < 8 lives in ccg[0], slot >= 8 in ccg[1];
                        # issue both candidate DMAs -- cond makes the
                        # wrong one skip itself at runtime (semaphore
                        # still fires), so exactly one lands.
                        peer = slotv % 8
                        g = []
                        for q in range(NQ):
                            gt = gp.tile([128, NQ, NL], FP8,
                                         tag=f"g{mv[0]}{q}",
                                         name=f"g{j}_{q}")
                            nc.sync.dma_start(gt[:], ccg[0][peer, q],
                                              cond=slotv < 8)
                            nc.sync.dma_start(gt[:], ccg[1][peer, q],
                                              cond=slotv >= 8)
                            g.append(gt)
                    stat = lf8[st]
                    if has_cs:
                        csa = accp.tile([128, 2, 512], BF16, tag="cs",
                                        bufs=3, name=f"cs{j}")
                    for itp in range(4):
                        ps = pp.tile([128, 4, 512], F32, tag="big",
                                     name=f"ps{j}_{itp}")
                        for sub in range(2):
                            it = itp * 2 + sub
                            lsl = slice(it * 128, it * 128 + 128)
                            for jhh in range(2):
                                for q in range(NQ):
                                    nc.tensor.matmul(
                                        ps[:, sub * 2 + jhh, :],
                                        stat[q][:, :, lsl],
                                        g[q][:, :,
                                             jhh * 512:(jhh + 1) * 512],
                                        start=q == 0, stop=q == NQ - 1,
                                        perf_mode=DR)
                        e = ep.tile([128, 4, 512], BF16, tag="e",
                                    name=f"e{j}_{itp}")
                        nc.scalar.activation(e[:], ps[:], AF.Exp,
                                             scale=SCALE_DEV)
                        for sub in range(2):
                            it = itp * 2 + sub
                            col = j * NIT + it
                            nc.vector.tensor_reduce(
                                rs_sb[:, col:col + 1],
                                e[:, sub * 2:sub * 2 + 2, :],
                                axis=mybir.AxisListType.XY, op=ALU.add)
                            if has_cs:
                                if itp == 0 and sub == 0:
                                    nc.gpsimd.tensor_copy(csa[:],
                                                          e[:, 0:2, :])
                                else:
                                    nc.gpsimd.tensor_tensor(
                                        csa[:], csa[:],
                                        e[:, sub * 2:sub * 2 + 2, :],
                                        op=ALU.add)
                    if has_cs:
                        psc = pp.tile([128, 4, 512], F32, tag="big",
                                      name=f"psc{j}")
                        cst = accp.tile([1, NL], F32, tag="csst", bufs=2,
                                        name=f"cst{j}")
                        for jhh in range(2):
                            nc.tensor.matmul(psc[0:1, jhh, :],
                                             ones_col_bf[:],
                                             csa[:, jhh, :],
                                             start=True, stop=True)
                            nc.vector.tensor_copy(
                                cst[:, jhh * 512:(jhh + 1) * 512],
                                psc[0:1, jhh, :])
                        nc.sync.dma_start(
                            out_cs[:, cs_slot * NL:(cs_slot + 1) * NL],
                            cst[:])
                        cs_slot += 1
                nc.sync.dma_start(out_rs[:, :], rs_sb[:])

    nc.compile()
    return nc


def _get_nc():
    if "nc" not in _CACHE:
        _CACHE["nc"] = _build()
    return _CACHE["nc"]


def kernel(z1, z2, index, fc1_w, fc1_b, fc2_w, fc2_b, **_unused):
    z1 = np.asarray(z1, np.float32)
    z2 = np.asarray(z2, np.float32)
    fc1_w = np.asarray(fc1_w, np.float32)
    fc1_b = np.asarray(fc1_b, np.float32)
    fc2_w = np.asarray(fc2_w, np.float32)
    fc2_b = np.asarray(fc2_b, np.float32)

    f8 = mybir.dt.np(FP8)
    bf = mybir.dt.np(BF16)

    def pack_dr(arr_t):  # [D, cols] -> [q, p, pair, cols] fp8
        d, cols = arr_t.shape
        a = arr_t.astype(f8).reshape(NQ, NQ, 128, cols).transpose(0, 2, 1, 3)
        return np.ascontiguousarray(a)

    z1t = np.ascontiguousarray(z1.T)  # [D, N]
    z2t = np.ascontiguousarray(z2.T)
    w1f8 = pack_dr(fc1_w * SIGMA)
    w2f8 = pack_dr(fc2_w * SIGMA)
    b1r = np.ascontiguousarray((SIGMA * fc1_b).reshape(1, D)).astype(bf)
    # fold the +1 shift of (elu+1) back out through layer 2
    b2r = np.ascontiguousarray(
        (SIGMA * (fc2_b - fc2_w.sum(axis=0))).reshape(1, D)).astype(bf)

    in_maps = []
    for r in range(NCORES):
        sl = slice(r * NL, (r + 1) * NL)
        in_maps.append({
            "z0f8": pack_dr(z1t[:, sl]),
            "z1f8": pack_dr(z2t[:, sl]),
            "w1f8": w1f8, "w2f8": w2f8, "b1r": b1r, "b2r": b2r,
        })

    nc = _get_nc()
    res = run_bass_kernel_spmd(nc, in_maps, list(range(NCORES)))

    E2 = np.exp(np.float64(1.0 / TAU))  # exp(2 * ||hn||^2), ||hn||^2 == 1
    rs_total = np.zeros(2 * N, np.float64)
    diag = np.zeros(N, np.float64)
    for k in range(NCORES):
        out = res.results[k]
        rs = out["out_rs"].astype(np.float64)       # [128, NBLK*8]
        cs = out["out_cs"].reshape(NCS, NL).astype(np.float64)
        diag[k * NL:(k + 1) * NL] = out["out_diag"].reshape(NL)
        slot = 0
        for j, (st, mv, has_cs) in enumerate(SCHED):
            rb = k + 8 * st
            rows = rs[:, j * NIT:(j + 1) * NIT]     # [128, 8]
            rs_total[rb * NL:(rb + 1) * NL] += rows.T.reshape(NL)
            if has_cs:
                if mv[0] == "local":
                    cb = k + 8
                elif mv[0] == "dynA":
                    cb = (k + mv[1]) % 16
                else:
                    cb = (k + 8 + mv[1]) % 16
                rs_total[cb * NL:(cb + 1) * NL] += cs[slot]
                slot += 1

    denom1 = rs_total[:N] - E2
    denom2 = rs_total[N:] - E2
    l_sum = 0.5 * (np.log(denom1) + np.log(denom2)) - (1.0 / TAU) * diag
    return np.float32(l_sum.mean() )


# revision 16
# speedup vs baseline: 1.8017x; 1.8017x over previous
"""Distributed GRACE-style contrastive loss on 8 Trainium2 NeuronCores.

Math (reference):
    h = elu(z @ W1 + b1) @ W2 + b2           for z1, z2    -> h1, h2
    hn = h / max(||h||_row, eps)
    With H = [h1n; h2n] (2N x D) and G = H H^T (symmetric, 2N x 2N):
      denom1_i = sum_j exp(2 G[i, j])     - e^2   (row i < N)
      denom2_i = sum_j exp(2 G[N+i, j])   - e^2
      loss = mean_i [ 0.5 (log denom1_i + log denom2_i) - 2 (h1n_i . h2n_i) ]

Strategy: G is symmetric, so only the upper triangle of its 16x16 grid of
1024x1024 blocks (136 blocks) is computed; each off-diagonal block yields
both a row-sum (DVE reduce of exp) and a column-sum (Pool-engine
accumulate + ones-matmul partition reduce) so every exp element serves
two output rows.  A circulant cover assigns exactly 17 blocks per core --
core k computes (k, k+d) and (k+8, (k+8+d) mod 16) for d = 1..7 plus the
three blocks touching only local data -- so the stationary matmul operand
is always the core's own fp8 tile and only the moving-tile DMA source
offsets depend on the rank (computed at runtime from partition_id).
Projection runs transposed (features on partitions) with fp8 DoubleRow
matmuls, biases folded in via rank-1 bias matmuls so the elu/normalize
chain reads PSUM directly.  Both normalized tensors AllGather into one
16-slot shared buffer (slot = global row block).  Final log/mean combine
runs on the host from small per-core outputs.
"""

import sys

sys.path.insert(0, "/opt/trn_rl_repo")

import numpy as np
from concourse import bacc, mybir, tile
from concourse.bass_utils import run_bass_kernel_spmd

F32 = mybir.dt.float32
BF16 = mybir.dt.bfloat16
FP8 = mybir.dt.float8e4
AF = mybir.ActivationFunctionType
ALU = mybir.AluOpType
DR = mybir.MatmulPerfMode.DoubleRow

N = 8192          # total rows per tensor
D = 512           # hidden dim (= proj dim)
NCORES = 8
NL = N // NCORES  # 1024 local rows per core per tensor
TAU = 0.5
SIGMA = 16.0      # fp8 pre-scale; S accumulates SIGMA^2 * S_true
SCALE_DEV = (1.0 / TAU) / (SIGMA * SIGMA)  # exp scale on device
NQ = 2            # two K=256 DoubleRow groups
NDC = D // 128    # 4 feature chunks of 128 partitions
NIT = NL // 128   # 8 row tiles of 128 per block
NSLOT = 16        # gathered row blocks (2 tensors x 8 cores)
EPS = 1e-12

# Per-core block schedule; identical structure on every core, the moving
# slot for dyn entries is rank-relative: dynA -> slot rank+d, dynB ->
# slot (rank+d+8) mod 16.  Blocks sharing a stationary tile are paired
# into units so one 2048-wide ACT exp with accum_out covers both blocks
# and yields their (shared-row) row-sum in a single accumulator read.
# unit = (stat_t, [(moving, has_colsum), ...])
UNITS = [
    (0, [(("local", 0), False), (("local", 1), True)]),  # (k,k), (k,k+8)
    (1, [(("local", 1), False), (("dynB", 1), True)]),   # (k+8,k+8), B1
    (0, [(("dynA", 1), True), (("dynA", 2), True)]),
    (1, [(("dynB", 2), True), (("dynB", 3), True)]),
    (0, [(("dynA", 3), True), (("dynA", 4), True)]),
    (1, [(("dynB", 4), True), (("dynB", 5), True)]),
    (0, [(("dynA", 5), True), (("dynA", 6), True)]),
    (1, [(("dynB", 6), True), (("dynB", 7), True)]),
    (0, [(("dynA", 7), True)]),
]
NUNIT = len(UNITS)                      # 9
NCS = sum(1 for u in UNITS for b in u[1] if b[1])   # 15

_CACHE = {}


def _build():
    nc = bacc.Bacc("TRN2", target_bir_lowering=False, debug=False,
                   num_devices=NCORES)

    # ---- I/O ----------------------------------------------------------
    # z comes pre-packed in fp8 DoubleRow pair layout [q, p, pair, cols]
    # with contraction index d = q*256 + pair*128 + p
    z_d = [nc.declare_dram_parameter(f"z{t}f8", [NQ, 128, NQ, NL], FP8,
                                     isOutput=False) for t in range(2)]
    w1_d = nc.declare_dram_parameter("w1f8", [NQ, 128, NQ, D], FP8,
                                     isOutput=False)
    w2_d = nc.declare_dram_parameter("w2f8", [NQ, 128, NQ, D], FP8,
                                     isOutput=False)
    # bias rows (x16, bf16) for the rank-1 bias matmuls; b2 adjusted on
    # host: b2 - colsum(W2) (folds the elu()+1 shift back out)
    b1_d = nc.declare_dram_parameter("b1r", [1, D], BF16, isOutput=False)
    b2_d = nc.declare_dram_parameter("b2r", [1, D], BF16, isOutput=False)

    out_rs = nc.declare_dram_parameter("out_rs", [128, NUNIT * NIT], F32,
                                       isOutput=True)
    out_cs = nc.declare_dram_parameter("out_cs", [1, NCS * NL], F32,
                                       isOutput=True)
    out_diag = nc.declare_dram_parameter("out_diag", [1, NL], F32,
                                         isOutput=True)

    with tile.TileContext(nc) as tc:
        with (
            tc.tile_pool(name="const", bufs=1) as constp,
            tc.tile_pool(name="locals", bufs=1) as localp,
            tc.tile_pool(name="accs", bufs=1) as accp,
            tc.tile_pool(name="dram", bufs=1, space="DRAM") as dramp,
        ):
            ones_col_bf = constp.tile([128, 1], BF16)
            nc.vector.memset(ones_col_bf[:], 1.0)
            ones_row_b = constp.tile([1, 512], BF16)
            nc.vector.memset(ones_row_b[:], 1.0)

            w1_sb = []
            w2_sb = []
            for q in range(NQ):
                w1t = constp.tile([128, NQ, D], FP8, name=f"w1_{q}")
                nc.sync.dma_start(w1t[:], w1_d[q])
                w1_sb.append(w1t)
                w2t = constp.tile([128, NQ, D], FP8, name=f"w2_{q}")
                nc.sync.dma_start(w2t[:], w2_d[q])
                w2_sb.append(w2t)
            b1_sb = constp.tile([1, D], BF16)
            nc.gpsimd.dma_start(b1_sb[:], b1_d[:])
            b2_sb = constp.tile([1, D], BF16)
            nc.gpsimd.dma_start(b2_sb[:], b2_d[:])

            # normalized local embeddings: bf16 [d, i] for the exact S12
            # diagonal, and x16-scaled fp8 in DoubleRow pair layout
            lns = [[localp.tile([128, NL], BF16, name=f"ln{t}_{dc}")
                    for dc in range(NDC)] for t in range(2)]
            lf8 = [[localp.tile([128, NQ, NL], FP8, name=f"lf8_{t}_{q}")
                    for q in range(NQ)] for t in range(2)]

            ccin = [dramp.tile([NQ, 128, NQ, NL], FP8, name=f"ccin{t}")
                    for t in range(2)]
            ccg = [dramp.tile([NCORES, NQ, 128, NQ, NL], FP8,
                              addr_space="Shared", name=f"ccg{t}")
                   for t in range(2)]

            rs_sb = accp.tile([128, NUNIT * NIT], F32)
            diag_sb = accp.tile([1, NL], F32)

            # ---- Phase A: projection + normalize ----------------------
            with (
                tc.tile_pool(name="zpool", bufs=2) as zp,
                tc.tile_pool(name="elupool", bufs=2) as elup,
                tc.tile_pool(name="hpool", bufs=2) as hp,
                tc.tile_pool(name="epool", bufs=6) as ep2,
                tc.tile_pool(name="rnpool", bufs=2) as rnp,
                tc.tile_pool(name="pa_ps", bufs=3, space="PSUM") as pp2,
                tc.tile_pool(name="pa_psn", bufs=1, space="PSUM") as psn,
            ):
                for t in range(2):
                    zq = []
                    zeng = nc.scalar if t == 0 else nc.gpsimd
                    for q in range(NQ):
                        z = zp.tile([128, NQ, NL], FP8, tag=f"z{q}",
                                    name=f"z{t}{q}")
                        zeng.dma_start(z[:], z_d[t][q])
                        zq.append(z)
                    elus = [elup.tile([128, NQ, NL], FP8, tag=f"el{q}",
                                      name=f"el{t}{q}") for q in range(NQ)]
                    ps_n = psn.tile([1, 2, 512], F32, tag="psn",
                                    name=f"psn{t}")
                    for ih in range(2):
                        isl = slice(ih * 512, ih * 512 + 512)
                        # layer 1 + exact elu+1 from PSUM:
                        # elu(x)+1 = relu(x) + min(exp(x), 1)
                        for q in range(NQ):
                            ps_a = pp2.tile([128, 2, 512], F32, tag="ps",
                                            name=f"psa{t}{ih}{q}")
                            for pair in range(2):
                                pc = q * 2 + pair
                                csl = slice(pc * 128, pc * 128 + 128)
                                nc.tensor.matmul(ps_a[:, pair, :],
                                                 b1_sb[:, csl],
                                                 ones_row_b[:],
                                                 start=True, stop=False)
                                for qq in range(NQ):
                                    nc.tensor.matmul(
                                        ps_a[:, pair, :],
                                        w1_sb[qq][:, :, csl],
                                        zq[qq][:, :, isl],
                                        start=False, stop=qq == NQ - 1,
                                        perf_mode=DR)
                            e_sb = ep2.tile([128, 2, 512], BF16, tag="pa",
                                            name=f"e{t}{ih}{q}")
                            nc.scalar.activation(e_sb[:], ps_a[:], AF.Exp,
                                                 scale=1.0 / SIGMA)
                            r_sb = ep2.tile([128, 2, 512], BF16, tag="pa",
                                            name=f"r{t}{ih}{q}")
                            nc.vector.tensor_scalar(
                                r_sb[:], ps_a[:], 0.0, 1.0 / SIGMA,
                                op0=ALU.max, op1=ALU.mult)
                            # elu(x)+1 = relu(x) + min(exp(x), 1)
                            nc.vector.scalar_tensor_tensor(
                                elus[q][:, :, isl], e_sb[:], 1.0, r_sb[:],
                                op0=ALU.min, op1=ALU.add)
                        # layer 2 + row norms
                        hts = []
                        for ocp in range(2):
                            ps_h = pp2.tile([128, 2, 512], F32, tag="ps",
                                            name=f"psh{t}{ih}{ocp}")
                            for pair in range(2):
                                oc = ocp * 2 + pair
                                csl = slice(oc * 128, oc * 128 + 128)
                                nc.tensor.matmul(ps_h[:, pair, :],
                                                 b2_sb[:, csl],
                                                 ones_row_b[:],
                                                 start=True, stop=False)
                                for qq in range(NQ):
                                    nc.tensor.matmul(
                                        ps_h[:, pair, :],
                                        w2_sb[qq][:, :, csl],
                                        elus[qq][:, :, isl],
                                        start=False, stop=qq == NQ - 1,
                                        perf_mode=DR)
                            # keep ACT to Exp/Sqrt only (table swaps are
                            # expensive); the 1/16 descale runs on DVE
                            ht = hp.tile([128, 2, 512], BF16,
                                         tag=f"h{ocp}", name=f"h{t}{ih}{ocp}")
                            nc.vector.tensor_scalar(ht[:], ps_h[:],
                                                    1.0 / SIGMA, None,
                                                    op0=ALU.mult)
                            sq = ep2.tile([128, 2, 512], BF16, tag="pa",
                                          name=f"sq{t}{ih}{ocp}")
                            nc.vector.tensor_tensor(sq[:], ht[:], ht[:],
                                                    op=ALU.mult)
                            for pair in range(2):
                                nc.tensor.matmul(
                                    ps_n[0:1, ih, :], ones_col_bf[:],
                                    sq[:, pair, :],
                                    start=ocp == 0 and pair == 0,
                                    stop=ocp == 1 and pair == 1)
                            hts.append(ht)
                        # 1/max(||h||, eps) per column, broadcast via
                        # rank-1 matmul
                        nm = rnp.tile([1, 512], F32, tag="nm",
                                      name=f"nm{t}{ih}")
                        nc.scalar.activation(nm[:], ps_n[0:1, ih, :],
                                             AF.Sqrt)
                        nm2 = rnp.tile([1, 512], F32, tag="nm2",
                                       name=f"nm2{t}{ih}")
                        nc.vector.tensor_scalar(nm2[:], nm[:], EPS, None,
                                                op0=ALU.max)
                        rn32 = rnp.tile([1, 512], F32, tag="rn32",
                                        name=f"rn32{t}{ih}")
                        nc.vector.reciprocal(rn32[:], nm2[:])
                        rn = rnp.tile([1, 512], BF16, tag="rn",
                                      name=f"rn{t}{ih}")
                        nc.vector.tensor_copy(rn[:], rn32[:])
                        ps_rb = pp2.tile([128, 2, 512], F32, tag="ps",
                                         name=f"psrb{t}{ih}")
                        nc.tensor.matmul(ps_rb[:, 0, :],
                                         ones_row_b[:, 0:128], rn[:],
                                         start=True, stop=True)
                        for oc in range(NDC):
                            ocp, pair = divmod(oc, 2)
                            nc.vector.tensor_tensor(
                                lns[t][oc][:, isl],
                                hts[ocp][:, pair, :], ps_rb[:, 0, :],
                                op=ALU.mult)
                            nc.vector.tensor_scalar(
                                lf8[t][oc // 2][:, oc % 2, isl],
                                lns[t][oc][:, isl], SIGMA, None,
                                op0=ALU.mult)
                    for q in range(NQ):
                        nc.sync.dma_start(ccin[t][q], lf8[t][q][:])
                    nc.gpsimd.collective_compute(
                        "AllGather", ALU.bypass,
                        replica_groups=[list(range(NCORES))],
                        ins=[ccin[t].opt()],
                        outs=[ccg[t].opt()],
                    )

                # ---- diag12[i] = h1n_i . h2n_i (local, bf16 exact) ----
                ps_d = psn.tile([1, 2, 512], F32, tag="psn", name="psd")
                for ihx in range(2):
                    isl = slice(ihx * 512, ihx * 512 + 512)
                    for dc in range(NDC):
                        pr = ep2.tile([128, 512], BF16, tag="pr",
                                      name=f"pr{ihx}{dc}")
                        nc.vector.tensor_tensor(pr[:], lns[0][dc][:, isl],
                                                lns[1][dc][:, isl],
                                                op=ALU.mult)
                        nc.tensor.matmul(ps_d[0:1, ihx, :], ones_col_bf[:],
                                         pr[:], start=dc == 0,
                                         stop=dc == NDC - 1)
                    nc.vector.tensor_copy(diag_sb[:, isl],
                                          ps_d[0:1, ihx, :])
                nc.sync.dma_start(out_diag[:, :], diag_sb[:])

            # ---- Phase C: 17 upper-triangle blocks of G, in 9 units ----
            with (
                tc.tile_pool(name="gpool", bufs=2) as gp,
                tc.tile_pool(name="ce", bufs=4) as ep,
                tc.tile_pool(name="pc_ps", bufs=2, space="PSUM") as pp,
            ):
                rank = nc.sync.partition_id()
                cs_slot = 0
                for u, (st, blocks) in enumerate(UNITS):
                    stat = lf8[st]
                    nb = len(blocks)
                    g = []        # per block: list of NQ moving tiles
                    csas = []     # per block: colsum accumulator or None
                    for bi, (mv, has_cs) in enumerate(blocks):
                        if mv[0] == "local":
                            g.append(lf8[mv[1]])
                        else:
                            d = mv[1]
                            if mv[0] == "dynA":
                                slotv = rank + d
                            else:
                                slotv = (rank + d + 8) % 16
                            # slot < 8 lives in ccg[0], slot >= 8 in
                            # ccg[1]; both candidate DMAs are issued and
                            # cond skips the wrong one at runtime (its
                            # semaphore still fires).
                            peer = slotv % 8
                            gt = []
                            for q in range(NQ):
                                gq = gp.tile([128, NQ, NL], FP8,
                                             tag=f"g{bi}_{q}",
                                             name=f"g{u}_{bi}{q}")
                                nc.sync.dma_start(gq[:], ccg[0][peer, q],
                                                  cond=slotv < 8)
                                nc.sync.dma_start(gq[:], ccg[1][peer, q],
                                                  cond=slotv >= 8)
                                gt.append(gq)
                            g.append(gt)
                        if has_cs:
                            csas.append(accp.tile([128, 2, 512], BF16,
                                                  tag=f"cs{bi}", bufs=2,
                                                  name=f"cs{u}_{bi}"))
                        else:
                            csas.append(None)
                    for it in range(NIT):
                        lsl = slice(it * 128, it * 128 + 128)
                        ps = pp.tile([128, 4, 512], F32, tag="big",
                                     name=f"ps{u}_{it}")
                        for bi in range(nb):
                            for jhh in range(2):
                                for q in range(NQ):
                                    nc.tensor.matmul(
                                        ps[:, bi * 2 + jhh, :],
                                        stat[q][:, :, lsl],
                                        g[bi][q][:, :,
                                                 jhh * 512:(jhh + 1) * 512],
                                        start=q == 0, stop=q == NQ - 1,
                                        perf_mode=DR)
                        e = ep.tile([128, 4, 512], BF16, tag="e",
                                    name=f"e{u}_{it}")
                        # one exp over the unit's full row-tile; accum_out
                        # is its row-sum (both blocks share rows)
                        nc.scalar.activation(
                            e[:, 0:2 * nb, :], ps[:, 0:2 * nb, :], AF.Exp,
                            scale=SCALE_DEV,
                            accum_out=rs_sb[:, u * NIT + it:
                                            u * NIT + it + 1])
                        for bi in range(nb):
                            if csas[bi] is None:
                                continue
                            esl = e[:, bi * 2:bi * 2 + 2, :]
                            if it == 0:
                                nc.vector.tensor_copy(csas[bi][:], esl)
                            else:
                                nc.vector.tensor_tensor(
                                    csas[bi][:], csas[bi][:], esl,
                                    op=ALU.add)
                    for bi in range(nb):
                        if csas[bi] is None:
                            continue
                        psc = pp.tile([128, 4, 512], F32, tag="big",
                                      name=f"psc{u}_{bi}")
                        cst = accp.tile([1, NL], F32, tag="csst", bufs=2,
                                        name=f"cst{u}_{bi}")
                        for jhh in range(2):
                            nc.tensor.matmul(psc[0:1, jhh, :],
                                             ones_col_bf[:],
                                             csas[bi][:, jhh, :],
                                             start=True, stop=True)
                            nc.vector.tensor_copy(
                                cst[:, jhh * 512:(jhh + 1) * 512],
                                psc[0:1, jhh, :])
                        nc.sync.dma_start(
                            out_cs[:, cs_slot * NL:(cs_slot + 1) * NL],
                            cst[:])
                        cs_slot += 1
                nc.sync.dma_start(out_rs[:, :], rs_sb[:])

    nc.compile()
    return nc


def _get_nc():
    if "nc" not in _CACHE:
        _CACHE["nc"] = _build()
    return _CACHE["nc"]


def kernel(z1, z2, index, fc1_w, fc1_b, fc2_w, fc2_b, **_unused):
    z1 = np.asarray(z1, np.float32)
    z2 = np.asarray(z2, np.float32)
    fc1_w = np.asarray(fc1_w, np.float32)
    fc1_b = np.asarray(fc1_b, np.float32)
    fc2_w = np.asarray(fc2_w, np.float32)
    fc2_b = np.asarray(fc2_b, np.float32)

    f8 = mybir.dt.np(FP8)
    bf = mybir.dt.np(BF16)

    def pack_dr(arr_t):  # [D, cols] -> [q, p, pair, cols] fp8
        d, cols = arr_t.shape
        a = arr_t.astype(f8).reshape(NQ, NQ, 128, cols).transpose(0, 2, 1, 3)
        return np.ascontiguousarray(a)

    z1t = np.ascontiguousarray(z1.T)  # [D, N]
    z2t = np.ascontiguousarray(z2.T)
    w1f8 = pack_dr(fc1_w * SIGMA)
    w2f8 = pack_dr(fc2_w * SIGMA)
    b1r = np.ascontiguousarray((SIGMA * fc1_b).reshape(1, D)).astype(bf)
    # fold the +1 shift of (elu+1) back out through layer 2
    b2r = np.ascontiguousarray(
        (SIGMA * (fc2_b - fc2_w.sum(axis=0))).reshape(1, D)).astype(bf)

    in_maps = []
    for r in range(NCORES):
        sl = slice(r * NL, (r + 1) * NL)
        in_maps.append({
            "z0f8": pack_dr(z1t[:, sl]),
            "z1f8": pack_dr(z2t[:, sl]),
            "w1f8": w1f8, "w2f8": w2f8, "b1r": b1r, "b2r": b2r,
        })

    nc = _get_nc()
    res = run_bass_kernel_spmd(nc, in_maps, list(range(NCORES)))

    E2 = np.exp(np.float64(1.0 / TAU))  # exp(2 * ||hn||^2), ||hn||^2 == 1
    rs_total = np.zeros(2 * N, np.float64)
    diag = np.zeros(N, np.float64)
    for k in range(NCORES):
        out = res.results[k]
        rs = out["out_rs"].astype(np.float64)       # [128, NUNIT*8]
        cs = out["out_cs"].reshape(NCS, NL).astype(np.float64)
        diag[k * NL:(k + 1) * NL] = out["out_diag"].reshape(NL)
        slot = 0
        for u, (st, blocks) in enumerate(UNITS):
            rb = k + 8 * st
            rows = rs[:, u * NIT:(u + 1) * NIT]     # [128, 8]
            rs_total[rb * NL:(rb + 1) * NL] += rows.T.reshape(NL)
            for mv, has_cs in blocks:
                if not has_cs:
                    continue
                if mv[0] == "local":
                    cb = k + 8
                elif mv[0] == "dynA":
                    cb = (k + mv[1]) % 16
                else:
                    cb = (k + 8 + mv[1]) % 16
                rs_total[cb * NL:(cb + 1) * NL] += cs[slot]
                slot += 1

    denom1 = rs_total[:N] - E2
    denom2 = rs_total[N:] - E2
    l_sum = 0.5 * (np.log(denom1) + np.log(denom2)) - (1.0 / TAU) * diag
    return np.float32(l_sum.mean() )


# revision 20
# speedup vs baseline: 1.9029x; 1.0562x over previous
"""Distributed GRACE-style contrastive loss on 8 Trainium2 NeuronCores.

Math (reference):
    h = elu(z @ W1 + b1) @ W2 + b2           for z1, z2    -> h1, h2
    hn = h / max(||h||_row, eps)
    With H = [h1n; h2n] (2N x D) and G = H H^T (symmetric, 2N x 2N):
      denom1_i = sum_j exp(2 G[i, j])     - e^2   (row i < N)
      denom2_i = sum_j exp(2 G[N+i, j])   - e^2
      loss = mean_i [ 0.5 (log denom1_i + log denom2_i) - 2 (h1n_i . h2n_i) ]

Strategy: G is symmetric, so only the upper triangle of its 16x16 grid of
1024x1024 blocks (136 blocks) is computed; each off-diagonal block yields
both a row-sum (DVE reduce of exp) and a column-sum (Pool-engine
accumulate + ones-matmul partition reduce) so every exp element serves
two output rows.  A circulant cover assigns exactly 17 blocks per core --
core k computes (k, k+d) and (k+8, (k+8+d) mod 16) for d = 1..7 plus the
three blocks touching only local data -- so the stationary matmul operand
is always the core's own fp8 tile and only the moving-tile DMA source
offsets depend on the rank (computed at runtime from partition_id).
Projection runs transposed (features on partitions) with fp8 DoubleRow
matmuls, biases folded in via rank-1 bias matmuls so the elu/normalize
chain reads PSUM directly.  Both normalized tensors AllGather into one
16-slot shared buffer (slot = global row block).  Final log/mean combine
runs on the host from small per-core outputs.
"""

import sys

sys.path.insert(0, "/opt/trn_rl_repo")

import numpy as np
from concourse import bacc, mybir, tile
from concourse.bass_utils import run_bass_kernel_spmd

F32 = mybir.dt.float32
BF16 = mybir.dt.bfloat16
FP8 = mybir.dt.float8e4
AF = mybir.ActivationFunctionType
ALU = mybir.AluOpType
DR = mybir.MatmulPerfMode.DoubleRow

N = 8192          # total rows per tensor
D = 512           # hidden dim (= proj dim)
NCORES = 8
NL = N // NCORES  # 1024 local rows per core per tensor
TAU = 0.5
SIGMA = 16.0      # fp8 pre-scale; S accumulates SIGMA^2 * S_true
SCALE_DEV = (1.0 / TAU) / (SIGMA * SIGMA)  # exp scale on device
NQ = 2            # two K=256 DoubleRow groups
NDC = D // 128    # 4 feature chunks of 128 partitions
NIT = NL // 128   # 8 row tiles of 128 per block
NSLOT = 16        # gathered row blocks (2 tensors x 8 cores)
EPS = 1e-12

# Per-core block schedule; identical structure on every core, the moving
# slot for dyn entries is rank-relative: dynA -> slot rank+d, dynB ->
# slot (rank+d+8) mod 16.  Blocks sharing a stationary tile are paired
# into units so one 2048-wide ACT exp with accum_out covers both blocks
# and yields their (shared-row) row-sum in a single accumulator read.
# unit = (stat_t, [(moving, has_colsum), ...])
UNITS = [
    (0, [(("local", 0), False)]),                      # (k,k): t0 only
    (1, [(("local", 1), False)]),                      # (k+8,k+8)
    (0, [(("local", 1), True)]),                       # (k,k+8)
    (0, [(("dynA", 1), True), (("dynA", 2), True)]),
    (1, [(("dynB", 1), True), (("dynB", 2), True)]),
    (0, [(("dynA", 3), True), (("dynA", 4), True)]),
    (1, [(("dynB", 3), True), (("dynB", 4), True)]),
    (0, [(("dynA", 5), True), (("dynA", 6), True)]),
    (1, [(("dynB", 5), True), (("dynB", 6), True)]),
    (0, [(("dynA", 7), True)]),
    (1, [(("dynB", 7), True)]),
]
NUNIT = len(UNITS)                      # 11
NCS = sum(1 for u in UNITS for b in u[1] if b[1])   # 15

_CACHE = {}


def _build():
    nc = bacc.Bacc("TRN2", target_bir_lowering=False, debug=False,
                   num_devices=NCORES)

    # ---- I/O ----------------------------------------------------------
    # z comes pre-packed in fp8 DoubleRow pair layout [q, p, pair, cols]
    # with contraction index d = q*256 + pair*128 + p
    z_d = [nc.declare_dram_parameter(f"z{t}f8", [NQ, 128, NQ, NL], FP8,
                                     isOutput=False) for t in range(2)]
    w1_d = nc.declare_dram_parameter("w1f8", [NQ, 128, NQ, D], FP8,
                                     isOutput=False)
    w2_d = nc.declare_dram_parameter("w2f8", [NQ, 128, NQ, D], FP8,
                                     isOutput=False)
    # bias rows (x16, bf16) for the rank-1 bias matmuls; b2 adjusted on
    # host: b2 - colsum(W2) (folds the elu()+1 shift back out)
    b1_d = nc.declare_dram_parameter("b1r", [1, D], BF16, isOutput=False)
    b2_d = nc.declare_dram_parameter("b2r", [1, D], BF16, isOutput=False)

    out_rs = nc.declare_dram_parameter("out_rs", [128, NUNIT * NIT], F32,
                                       isOutput=True)
    out_cs = nc.declare_dram_parameter("out_cs", [1, NCS * NL], F32,
                                       isOutput=True)
    out_diag = nc.declare_dram_parameter("out_diag", [1, NL], F32,
                                         isOutput=True)

    with tile.TileContext(nc) as tc:
        with (
            tc.tile_pool(name="const", bufs=1) as constp,
            tc.tile_pool(name="locals", bufs=1) as localp,
            tc.tile_pool(name="accs", bufs=1) as accp,
            tc.tile_pool(name="dram", bufs=1, space="DRAM") as dramp,
        ):
            ones_col_bf = constp.tile([128, 1], BF16)
            nc.vector.memset(ones_col_bf[:], 1.0)
            ones_row_b = constp.tile([1, 512], BF16)
            nc.vector.memset(ones_row_b[:], 1.0)

            w1_sb = []
            w2_sb = []
            for q in range(NQ):
                w1t = constp.tile([128, NQ, D], FP8, name=f"w1_{q}")
                nc.sync.dma_start(w1t[:], w1_d[q])
                w1_sb.append(w1t)
                w2t = constp.tile([128, NQ, D], FP8, name=f"w2_{q}")
                nc.sync.dma_start(w2t[:], w2_d[q])
                w2_sb.append(w2t)
            b1_sb = constp.tile([1, D], BF16)
            nc.gpsimd.dma_start(b1_sb[:], b1_d[:])
            b2_sb = constp.tile([1, D], BF16)
            nc.gpsimd.dma_start(b2_sb[:], b2_d[:])

            # normalized local embeddings: bf16 [d, i] for the exact S12
            # diagonal, and x16-scaled fp8 in DoubleRow pair layout
            lns = [[localp.tile([128, NL], BF16, name=f"ln{t}_{dc}")
                    for dc in range(NDC)] for t in range(2)]
            lf8 = [[localp.tile([128, NQ, NL], FP8, name=f"lf8_{t}_{q}")
                    for q in range(NQ)] for t in range(2)]

            ccin = [dramp.tile([NQ, 128, NQ, NL], FP8, name=f"ccin{t}")
                    for t in range(2)]
            ccg = [dramp.tile([NCORES, NQ, 128, NQ, NL], FP8,
                              addr_space="Shared", name=f"ccg{t}")
                   for t in range(2)]

            rs_sb = accp.tile([128, NUNIT * NIT], F32)
            diag_sb = accp.tile([1, NL], F32)

            # ---- Phase A: projection + normalize ----------------------
            with (
                tc.tile_pool(name="zpool", bufs=2) as zp,
                tc.tile_pool(name="elupool", bufs=2) as elup,
                tc.tile_pool(name="hpool", bufs=2) as hp,
                tc.tile_pool(name="epool", bufs=6) as ep2,
                tc.tile_pool(name="rnpool", bufs=2) as rnp,
                tc.tile_pool(name="pa_ps", bufs=3, space="PSUM") as pp2,
                tc.tile_pool(name="pa_psn", bufs=1, space="PSUM") as psn,
            ):
                for t in range(2):
                    zq = []
                    zeng = nc.scalar if t == 0 else nc.gpsimd
                    for q in range(NQ):
                        z = zp.tile([128, NQ, NL], FP8, tag=f"z{q}",
                                    name=f"z{t}{q}")
                        zeng.dma_start(z[:], z_d[t][q])
                        zq.append(z)
                    elus = [elup.tile([128, NQ, NL], FP8, tag=f"el{q}",
                                      name=f"el{t}{q}") for q in range(NQ)]
                    ps_n = psn.tile([1, 2, 512], F32, tag="psn",
                                    name=f"psn{t}")
                    for ih in range(2):
                        isl = slice(ih * 512, ih * 512 + 512)
                        # layer 1 + exact elu+1 from PSUM:
                        # elu(x)+1 = relu(x) + min(exp(x), 1)
                        for q in range(NQ):
                            ps_a = pp2.tile([128, 2, 512], F32, tag="ps",
                                            name=f"psa{t}{ih}{q}")
                            for pair in range(2):
                                pc = q * 2 + pair
                                csl = slice(pc * 128, pc * 128 + 128)
                                nc.tensor.matmul(ps_a[:, pair, :],
                                                 b1_sb[:, csl],
                                                 ones_row_b[:],
                                                 start=True, stop=False)
                                for qq in range(NQ):
                                    nc.tensor.matmul(
                                        ps_a[:, pair, :],
                                        w1_sb[qq][:, :, csl],
                                        zq[qq][:, :, isl],
                                        start=False, stop=qq == NQ - 1,
                                        perf_mode=DR)
                            e_sb = ep2.tile([128, 2, 512], BF16, tag="pa",
                                            name=f"e{t}{ih}{q}")
                            nc.scalar.activation(e_sb[:], ps_a[:], AF.Exp,
                                                 scale=1.0 / SIGMA)
                            # relu is present in every ACT table -> free
                            r_sb = ep2.tile([128, 2, 512], BF16, tag="pa",
                                            name=f"r{t}{ih}{q}")
                            nc.scalar.activation(r_sb[:], ps_a[:], AF.Relu,
                                                 scale=1.0 / SIGMA)
                            # elu(x)+1 = relu(x) + min(exp(x), 1)
                            nc.vector.scalar_tensor_tensor(
                                elus[q][:, :, isl], e_sb[:], 1.0, r_sb[:],
                                op0=ALU.min, op1=ALU.add)
                        # layer 2 + row norms
                        hts = []
                        for ocp in range(2):
                            ps_h = pp2.tile([128, 2, 512], F32, tag="ps",
                                            name=f"psh{t}{ih}{ocp}")
                            for pair in range(2):
                                oc = ocp * 2 + pair
                                csl = slice(oc * 128, oc * 128 + 128)
                                nc.tensor.matmul(ps_h[:, pair, :],
                                                 b2_sb[:, csl],
                                                 ones_row_b[:],
                                                 start=True, stop=False)
                                for qq in range(NQ):
                                    nc.tensor.matmul(
                                        ps_h[:, pair, :],
                                        w2_sb[qq][:, :, csl],
                                        elus[qq][:, :, isl],
                                        start=False, stop=qq == NQ - 1,
                                        perf_mode=DR)
                            # Copy is present in every ACT table -> free
                            ht = hp.tile([128, 2, 512], BF16,
                                         tag=f"h{ocp}", name=f"h{t}{ih}{ocp}")
                            nc.scalar.activation(ht[:], ps_h[:], AF.Copy,
                                                 scale=1.0 / SIGMA)
                            sq = ep2.tile([128, 2, 512], BF16, tag="pa",
                                          name=f"sq{t}{ih}{ocp}")
                            nc.vector.tensor_tensor(sq[:], ht[:], ht[:],
                                                    op=ALU.mult)
                            for pair in range(2):
                                nc.tensor.matmul(
                                    ps_n[0:1, ih, :], ones_col_bf[:],
                                    sq[:, pair, :],
                                    start=ocp == 0 and pair == 0,
                                    stop=ocp == 1 and pair == 1)
                            hts.append(ht)
                        # 1/max(||h||, eps) = exp(-0.5 ln(max(||h||^2,
                        # eps^2))); Ln and Exp share an ACT table so the
                        # whole kernel runs without table swaps
                        nm2 = rnp.tile([1, 512], F32, tag="nm2",
                                       name=f"nm2{t}{ih}")
                        nc.vector.tensor_scalar(nm2[:], ps_n[0:1, ih, :],
                                                EPS * EPS, None,
                                                op0=ALU.max)
                        lnv = rnp.tile([1, 512], F32, tag="lnv",
                                       name=f"lnv{t}{ih}")
                        nc.scalar.activation(lnv[:], nm2[:], AF.Ln)
                        rn = rnp.tile([1, 512], BF16, tag="rn",
                                      name=f"rn{t}{ih}")
                        nc.scalar.activation(rn[:], lnv[:], AF.Exp,
                                             scale=-0.5)
                        ps_rb = pp2.tile([128, 2, 512], F32, tag="ps",
                                         name=f"psrb{t}{ih}")
                        nc.tensor.matmul(ps_rb[:, 0, :],
                                         ones_row_b[:, 0:128], rn[:],
                                         start=True, stop=True)
                        for oc in range(NDC):
                            ocp, pair = divmod(oc, 2)
                            nc.vector.tensor_tensor(
                                lns[t][oc][:, isl],
                                hts[ocp][:, pair, :], ps_rb[:, 0, :],
                                op=ALU.mult)
                            nc.vector.tensor_scalar(
                                lf8[t][oc // 2][:, oc % 2, isl],
                                lns[t][oc][:, isl], SIGMA, None,
                                op0=ALU.mult)
                    for q in range(NQ):
                        nc.sync.dma_start(ccin[t][q], lf8[t][q][:])
                    nc.gpsimd.collective_compute(
                        "AllGather", ALU.bypass,
                        replica_groups=[list(range(NCORES))],
                        ins=[ccin[t].opt()],
                        outs=[ccg[t].opt()],
                    )

                # ---- diag12[i] = h1n_i . h2n_i (local, bf16 exact) ----
                ps_d = psn.tile([1, 2, 512], F32, tag="psn", name="psd")
                for ihx in range(2):
                    isl = slice(ihx * 512, ihx * 512 + 512)
                    for dc in range(NDC):
                        pr = ep2.tile([128, 512], BF16, tag="pr",
                                      name=f"pr{ihx}{dc}")
                        nc.vector.tensor_tensor(pr[:], lns[0][dc][:, isl],
                                                lns[1][dc][:, isl],
                                                op=ALU.mult)
                        nc.tensor.matmul(ps_d[0:1, ihx, :], ones_col_bf[:],
                                         pr[:], start=dc == 0,
                                         stop=dc == NDC - 1)
                    nc.vector.tensor_copy(diag_sb[:, isl],
                                          ps_d[0:1, ihx, :])
                nc.sync.dma_start(out_diag[:, :], diag_sb[:])

            # ---- Phase C: 17 upper-triangle blocks of G, in 9 units ----
            with (
                tc.tile_pool(name="gpool", bufs=2) as gp,
                tc.tile_pool(name="ce", bufs=4) as ep,
                tc.tile_pool(name="pc_ps", bufs=2, space="PSUM") as pp,
            ):
                rank = nc.sync.partition_id()
                cs_slot = 0
                for u, (st, blocks) in enumerate(UNITS):
                    stat = lf8[st]
                    nb = len(blocks)
                    g = []        # per block: list of NQ moving tiles
                    csas = []     # per block: colsum accumulator or None
                    for bi, (mv, has_cs) in enumerate(blocks):
                        if mv[0] == "local":
                            g.append(lf8[mv[1]])
                        else:
                            d = mv[1]
                            if mv[0] == "dynA":
                                slotv = rank + d
                            else:
                                slotv = (rank + d + 8) % 16
                            # slot < 8 lives in ccg[0], slot >= 8 in
                            # ccg[1]; both candidate DMAs are issued and
                            # cond skips the wrong one at runtime (its
                            # semaphore still fires).
                            peer = slotv % 8
                            gt = []
                            for q in range(NQ):
                                gq = gp.tile([128, NQ, NL], FP8,
                                             tag=f"g{bi}_{q}",
                                             name=f"g{u}_{bi}{q}")
                                nc.sync.dma_start(gq[:], ccg[0][peer, q],
                                                  cond=slotv < 8)
                                nc.sync.dma_start(gq[:], ccg[1][peer, q],
                                                  cond=slotv >= 8)
                                gt.append(gq)
                            g.append(gt)
                        if has_cs:
                            csas.append(accp.tile([128, 2, 512], BF16,
                                                  tag=f"cs{bi}", bufs=2,
                                                  name=f"cs{u}_{bi}"))
                        else:
                            csas.append(None)
                    for it in range(NIT):
                        lsl = slice(it * 128, it * 128 + 128)
                        ps = pp.tile([128, 4, 512], F32, tag="big",
                                     name=f"ps{u}_{it}")
                        for bi in range(nb):
                            for jhh in range(2):
                                for q in range(NQ):
                                    nc.tensor.matmul(
                                        ps[:, bi * 2 + jhh, :],
                                        stat[q][:, :, lsl],
                                        g[bi][q][:, :,
                                                 jhh * 512:(jhh + 1) * 512],
                                        start=q == 0, stop=q == NQ - 1,
                                        perf_mode=DR)
                        e = ep.tile([128, 4, 512], BF16, tag="e",
                                    name=f"e{u}_{it}")
                        # one exp over the unit's full row-tile; accum_out
                        # is its row-sum (both blocks share rows)
                        nc.scalar.activation(
                            e[:, 0:2 * nb, :], ps[:, 0:2 * nb, :], AF.Exp,
                            scale=SCALE_DEV,
                            accum_out=rs_sb[:, u * NIT + it:
                                            u * NIT + it + 1])
                        for bi in range(nb):
                            if csas[bi] is None:
                                continue
                            esl = e[:, bi * 2:bi * 2 + 2, :]
                            if it == 0:
                                nc.vector.tensor_copy(csas[bi][:], esl)
                            else:
                                nc.vector.tensor_tensor(
                                    csas[bi][:], csas[bi][:], esl,
                                    op=ALU.add)
                    for bi in range(nb):
                        if csas[bi] is None:
                            continue
                        psc = pp.tile([128, 4, 512], F32, tag="big",
                                      name=f"psc{u}_{bi}")
                        cst = accp.tile([1, NL], F32, tag="csst", bufs=2,
                                        name=f"cst{u}_{bi}")
                        for jhh in range(2):
                            nc.tensor.matmul(psc[0:1, jhh, :],
                                             ones_col_bf[:],
                                             csas[bi][:, jhh, :],
                                             start=True, stop=True)
                            nc.vector.tensor_copy(
                                cst[:, jhh * 512:(jhh + 1) * 512],
                                psc[0:1, jhh, :])
                        nc.sync.dma_start(
                            out_cs[:, cs_slot * NL:(cs_slot + 1) * NL],
                            cst[:])
                        cs_slot += 1
                nc.sync.dma_start(out_rs[:, :], rs_sb[:])

    nc.compile()
    return nc


def _get_nc():
    if "nc" not in _CACHE:
        _CACHE["nc"] = _build()
    return _CACHE["nc"]


def kernel(z1, z2, index, fc1_w, fc1_b, fc2_w, fc2_b, **_unused):
    z1 = np.asarray(z1, np.float32)
    z2 = np.asarray(z2, np.float32)
    fc1_w = np.asarray(fc1_w, np.float32)
    fc1_b = np.asarray(fc1_b, np.float32)
    fc2_w = np.asarray(fc2_w, np.float32)
    fc2_b = np.asarray(fc2_b, np.float32)

    f8 = mybir.dt.np(FP8)
    bf = mybir.dt.np(BF16)

    def pack_dr(arr_t):  # [D, cols] -> [q, p, pair, cols] fp8
        d, cols = arr_t.shape
        a = arr_t.astype(f8).reshape(NQ, NQ, 128, cols).transpose(0, 2, 1, 3)
        return np.ascontiguousarray(a)

    z1t = np.ascontiguousarray(z1.T)  # [D, N]
    z2t = np.ascontiguousarray(z2.T)
    w1f8 = pack_dr(fc1_w * SIGMA)
    w2f8 = pack_dr(fc2_w * SIGMA)
    b1r = np.ascontiguousarray((SIGMA * fc1_b).reshape(1, D)).astype(bf)
    # fold the +1 shift of (elu+1) back out through layer 2
    b2r = np.ascontiguousarray(
        (SIGMA * (fc2_b - fc2_w.sum(axis=0))).reshape(1, D)).astype(bf)

    in_maps = []
    for r in range(NCORES):
        sl = slice(r * NL, (r + 1) * NL)
        in_maps.append({
            "z0f8": pack_dr(z1t[:, sl]),
            "z1f8": pack_dr(z2t[:, sl]),
            "w1f8": w1f8, "w2f8": w2f8, "b1r": b1r, "b2r": b2r,
        })

    nc = _get_nc()
    res = run_bass_kernel_spmd(nc, in_maps, list(range(NCORES)))

    E2 = np.exp(np.float64(1.0 / TAU))  # exp(2 * ||hn||^2), ||hn||^2 == 1
    rs_total = np.zeros(2 * N, np.float64)
    diag = np.zeros(N, np.float64)
    for k in range(NCORES):
        out = res.results[k]
        rs = out["out_rs"].astype(np.float64)       # [128, NUNIT*8]
        cs = out["out_cs"].reshape(NCS, NL).astype(np.float64)
        diag[k * NL:(k + 1) * NL] = out["out_diag"].reshape(NL)
        slot = 0
        for u, (st, blocks) in enumerate(UNITS):
            rb = k + 8 * st
            rows = rs[:, u * NIT:(u + 1) * NIT]     # [128, 8]
            rs_total[rb * NL:(rb + 1) * NL] += rows.T.reshape(NL)
            for mv, has_cs in blocks:
                if not has_cs:
                    continue
                if mv[0] == "local":
                    cb = k + 8
                elif mv[0] == "dynA":
                    cb = (k + mv[1]) % 16
                else:
                    cb = (k + 8 + mv[1]) % 16
                rs_total[cb * NL:(cb + 1) * NL] += cs[slot]
                slot += 1

    denom1 = rs_total[:N] - E2
    denom2 = rs_total[N:] - E2
    l_sum = 0.5 * (np.log(denom1) + np.log(denom2)) - (1.0 / TAU) * diag
    return np.float32(l_sum.mean() )
